# revision 14
# baseline (speedup 1.0000x reference)
"""Multi-head dense attention (no softmax) on 8 Trainium2 NeuronCores.

Math (per batch b, head h with head_dim d=64):
    q   = x @ W^T                      # [S, H] projection
    out_h = q_h (x_h^T x_h)            # Gram reassociation (exact)

Sharding: core c handles batch b = c//2 and head-group hg = c%2 (8 heads,
512 output columns). Cores are fully independent (no collectives).

v3 structure:
  - Phase 1: m-tiles 0+1 projected together, k-outer interleaved across
    8 PSUM banks, so the PE consumes each arriving xT k-tile at ~1.7us
    while the DMA stream delivers one every ~1.5us -> no idle ramp.
  - Phase 2 (4 freed banks + 2 gram + 2 out): mt2, gram, out0/1, mt3,
    out2/3 interleaved so output-stage drains overlap projection.
  - Gram runs on fp8(e4m3) xn with DoubleRow packing: 2 s-tiles per
    matmul (K=256), 32 matmuls instead of 64.
  - Input DMAs: SP queue carries wT-mtile0 + xT k-tiles in consumption
    order; Pool queue carries wT-rest + xn8; stores alternate SP/Pool.
  - PSUM drains alternate DVE/ACT; a warmup matmul burst keeps the PE
    clock ramped (HAM) during the initial DMA wait.
"""

import numpy as np

B, S, H = 4, 2048, 1024
N_HEADS = 16
HD = H // N_HEADS  # 64
N_CORES = 8
MG = H // 2        # 512 output columns per core
P = 128
KT = H // P        # 8 k-tiles
ST = S // P        # 16 s-tiles
SP2 = ST // 2      # 8 DoubleRow s-pairs
MT = MG // P       # 4 m-tiles == head pairs
SC = S // 512      # 4 s-chunks
W_SCALE = 1024.0
N_WARM = 10

_NC_CACHE = {}


def _build_nc():
    import concourse.mybir as mybir
    from concourse import bacc
    from concourse.tile import TileContext

    f32 = mybir.dt.float32
    f16 = mybir.dt.float16
    f8 = mybir.dt.float8e4

    nc = bacc.Bacc()
    xT_d = nc.declare_dram_parameter("xT", [H, S], f16, isOutput=False)
    xn_d = nc.declare_dram_parameter("xn8", [S, MG], f8, isOutput=False)
    wT_d = nc.declare_dram_parameter("wT", [MT * P, KT * P], f16, isOutput=False)
    outT_d = nc.declare_dram_parameter("outT", [MG, S], f16, isOutput=True)

    xT_t = xT_d.rearrange("(kt p) s -> p kt s", p=P)   # [128, 8, 2048]
    xn_t = xn_d.rearrange("(t i p) m -> p t i m", i=2, p=P)  # [128, 8, 2, 512]
    wT_t = wT_d.rearrange("(mt p) (kt m) -> p mt kt m", p=P, m=P)  # [128, 4, 8, 128]

    with TileContext(nc) as tc:
        with (
            tc.tile_pool(name="big", bufs=1) as big,
            tc.tile_pool(name="gp", bufs=1) as gpool,
            tc.tile_pool(name="stage", bufs=8) as stage,
        ):
            xT_sb = big.tile([P, KT, S], f16, tag="xT")
            xn_sb = big.tile([P, SP2, 2, MG], f8, tag="xn")
            wT_sb = big.tile([P, MT, KT, P], f16, tag="wT")
            qT_sb = big.tile([P, MT, S], f16, tag="qT")
            warm_sb = big.tile([P, 512], f16, tag="warm")

            # Pool-queue setup: warmup operand, zeroed G holders, then the
            # non-critical loads (wT m-tiles 1-3, xn8).
            nc.gpsimd.memset(warm_sb, 0.0)
            gbd = []
            for p_i in range(MT):
                g = gpool.tile([P, P], f16, tag=f"g{p_i}", name=f"g{p_i}")
                nc.gpsimd.memset(g, 0.0)
                gbd.append(g)
            # SP queue: everything in strict consumption order so nothing
            # steals HBM bandwidth from the PE-critical xT stream.
            # Minimal first slices so the PE's first real matmul starts as
            # early as possible (phase 1 is consumption-limited from then on).
            nc.sync.dma_start(out=wT_sb[:, 0, 0], in_=wT_t[:, 0, 0])
            nc.sync.dma_start(out=xT_sb[:, 0, 0:1024], in_=xT_t[:, 0, 0:1024])
            nc.sync.dma_start(out=wT_sb[:, 1, 0], in_=wT_t[:, 1, 0])
            nc.sync.dma_start(out=xT_sb[:, 0, 1024:], in_=xT_t[:, 0, 1024:])
            nc.sync.dma_start(out=wT_sb[:, 0, 1:], in_=wT_t[:, 0, 1:])
            nc.sync.dma_start(out=wT_sb[:, 1, 1:], in_=wT_t[:, 1, 1:])
            for kt in range(1, KT):
                nc.sync.dma_start(out=xT_sb[:, kt], in_=xT_t[:, kt])
            nc.sync.dma_start(out=xn_sb, in_=xn_t)
            nc.sync.dma_start(out=wT_sb[:, 2:], in_=wT_t[:, 2:])

            def drain_q(mt, psqs):
                for sc in range(SC):
                    dr = nc.vector if sc % 2 == 0 else nc.scalar
                    o = qT_sb[:, mt, sc * 512:(sc + 1) * 512]
                    if dr is nc.vector:
                        dr.tensor_copy(out=o, in_=psqs[sc])
                    else:
                        dr.copy(out=o, in_=psqs[sc])

            def emit_gram(ps_g):
                for p_i in range(MT):
                    psg = ps_g.tile([P, P], f32, tag="psg", name=f"psg{p_i}")
                    cols = slice(p_i * P, (p_i + 1) * P)
                    for t in range(SP2):
                        nc.tensor.matmul(
                            psg,
                            lhsT=xn_sb[:, t, :, cols],
                            rhs=xn_sb[:, t, :, cols],
                            start=(t == 0),
                            stop=(t == SP2 - 1),
                            perf_mode=mybir.MatmulPerfMode.DoubleRow,
                        )
                    nc.scalar.mul(
                        out=gbd[p_i][0:HD, 0:HD], in_=psg[0:HD, 0:HD],
                        mul=1.0 / W_SCALE,
                    )
                    nc.scalar.mul(
                        out=gbd[p_i][HD:P, HD:P], in_=psg[HD:P, HD:P],
                        mul=1.0 / W_SCALE,
                    )

            def emit_out_sc(p_i, sc, ps_o):
                pso = ps_o.tile([P, 512], f32, tag="pso", name=f"pso{p_i}_{sc}")
                nc.tensor.matmul(
                    pso,
                    lhsT=gbd[p_i],
                    rhs=qT_sb[:, p_i, sc * 512:(sc + 1) * 512],
                    start=True,
                    stop=True,
                )
                ot = stage.tile([P, 512], f16, tag="ot", name=f"ot{p_i}_{sc}")
                dr = nc.vector if sc % 2 == 1 else nc.scalar
                if dr is nc.vector:
                    dr.tensor_copy(out=ot, in_=pso)
                else:
                    dr.copy(out=ot, in_=pso)
                st = nc.sync if sc % 2 == 0 else nc.gpsimd
                st.dma_start(
                    out=outT_d[p_i * P:(p_i + 1) * P, sc * 512:(sc + 1) * 512],
                    in_=ot,
                )

            def emit_out(p_i, ps_o):
                for sc in range(SC):
                    emit_out_sc(p_i, sc, ps_o)

            # ---- Phase 1: mt0+mt1 k-outer interleaved over 8 PSUM banks.
            with tc.tile_pool(name="ps1", bufs=2, space="PSUM") as ps1:
                psq01 = [
                    [
                        ps1.tile([P, 512], f32, tag=f"psq{sc}", name=f"psq{mt}_{sc}")
                        for sc in range(SC)
                    ]
                    for mt in range(2)
                ]
                for i in range(N_WARM):
                    nc.tensor.matmul(
                        psq01[0][0],
                        lhsT=warm_sb[:, 0:P],
                        rhs=warm_sb,
                        start=True,
                        stop=True,
                    )
                for kt in range(KT):
                    for mt in range(2):
                        for sc in range(SC):
                            nc.tensor.matmul(
                                psq01[mt][sc],
                                lhsT=wT_sb[:, mt, kt],
                                rhs=xT_sb[:, kt, sc * 512:(sc + 1) * 512],
                                start=(kt == 0),
                                stop=(kt == KT - 1),
                            )
                drain_q(0, psq01[0])
                drain_q(1, psq01[1])

            # ---- Phase 2: gram first, then mt2/mt3 projected sc-major with
            # per-sc drain -> output-matmul -> store chains so the output
            # stage pipelines behind the projection instead of trailing it.
            with (
                tc.tile_pool(name="ps2", bufs=1, space="PSUM") as ps2,
                tc.tile_pool(name="ps_g", bufs=2, space="PSUM") as ps_g,
                tc.tile_pool(name="ps_o", bufs=2, space="PSUM") as ps_o,
            ):
                emit_gram(ps_g)
                emit_out(0, ps_o)
                emit_out(1, ps_o)
                for mt in (2, 3):
                    for sc in range(SC):
                        psq = ps2.tile(
                            [P, 512], f32, tag=f"psq{sc}", name=f"psq{mt}_{sc}"
                        )
                        for kt in range(KT):
                            nc.tensor.matmul(
                                psq,
                                lhsT=wT_sb[:, mt, kt],
                                rhs=xT_sb[:, kt, sc * 512:(sc + 1) * 512],
                                start=(kt == 0),
                                stop=(kt == KT - 1),
                            )
                        dr = nc.vector if sc % 2 == 1 else nc.scalar
                        o = qT_sb[:, mt, sc * 512:(sc + 1) * 512]
                        if dr is nc.vector:
                            dr.tensor_copy(out=o, in_=psq)
                        else:
                            dr.copy(out=o, in_=psq)
                        emit_out_sc(mt, sc, ps_o)
    nc.compile()
    return nc


def _get_nc():
    if "nc" not in _NC_CACHE:
        _NC_CACHE["nc"] = _build_nc()
    return _NC_CACHE["nc"]


def make_in_maps(hidden_states, queries_weight):
    import ml_dtypes

    hs = np.ascontiguousarray(np.asarray(hidden_states, dtype=np.float32))
    w = np.ascontiguousarray(np.asarray(queries_weight, dtype=np.float32))
    in_maps = []
    for c in range(N_CORES):
        b, hg = divmod(c, 2)
        xb = hs[b]
        in_maps.append({
            "xT": np.ascontiguousarray(xb.T).astype(np.float16),
            "xn8": np.ascontiguousarray(xb[:, hg * MG:(hg + 1) * MG]).astype(
                ml_dtypes.float8_e4m3fn
            ),
            "wT": np.ascontiguousarray(
                (w[hg * MG:(hg + 1) * MG, :].T * W_SCALE)
                .reshape(KT, P, MT, P)
                .transpose(2, 1, 0, 3)
                .reshape(MT * P, KT * P)
            ).astype(np.float16),
        })
    return in_maps


def assemble_output(results):
    out = np.empty((B, S, H), dtype=np.float32)
    for c in range(N_CORES):
        b, hg = divmod(c, 2)
        out[b, :, hg * MG:(hg + 1) * MG] = results[c]["outT"].T.astype(np.float32)
    return out


def kernel(hidden_states, queries_weight):
    from concourse.bass_utils import run_bass_kernel_spmd

    in_maps = make_in_maps(hidden_states, queries_weight)
    res = run_bass_kernel_spmd(
        _get_nc(), in_maps, core_ids=list(range(N_CORES))
    ).results
    return assemble_output(res)


if __name__ == "__main__":
    x = np.random.randn(B, S, H).astype(np.float32)
    w = np.random.randn(H, H).astype(np.float32) * 1e-4
    out = kernel(x, w)
    print(out.shape, out.dtype)


# revision 15
# speedup vs baseline: 1.0646x; 1.0646x over previous
"""Multi-head dense attention (no softmax) on 8 Trainium2 NeuronCores.

Math (per batch b, head h with head_dim d=64):
    q   = x @ W^T                      # [S, H] projection
    out_h = q_h (x_h^T x_h)            # Gram reassociation (exact)

Sharding: core c handles batch b = c//2 and head-group hg = c%2 (8 heads,
512 output columns). Cores are fully independent (no collectives).

v3 structure:
  - Phase 1: m-tiles 0+1 projected together, k-outer interleaved across
    8 PSUM banks, so the PE consumes each arriving xT k-tile at ~1.7us
    while the DMA stream delivers one every ~1.5us -> no idle ramp.
  - Phase 2 (4 freed banks + 2 gram + 2 out): mt2, gram, out0/1, mt3,
    out2/3 interleaved so output-stage drains overlap projection.
  - Gram runs on fp8(e4m3) xn with DoubleRow packing: 2 s-tiles per
    matmul (K=256), 32 matmuls instead of 64.
  - Input DMAs: SP queue carries wT-mtile0 + xT k-tiles in consumption
    order; Pool queue carries wT-rest + xn8; stores alternate SP/Pool.
  - PSUM drains alternate DVE/ACT; a warmup matmul burst keeps the PE
    clock ramped (HAM) during the initial DMA wait.
"""

import numpy as np

B, S, H = 4, 2048, 1024
N_HEADS = 16
HD = H // N_HEADS  # 64
N_CORES = 8
MG = H // 2        # 512 output columns per core
P = 128
KT = H // P        # 8 k-tiles
ST = S // P        # 16 s-tiles
SP2 = ST // 2      # 8 DoubleRow s-pairs
MT = MG // P       # 4 m-tiles == head pairs
SC = S // 512      # 4 s-chunks
W_SCALE = 1024.0
N_WARM = 13

_NC_CACHE = {}


def _build_nc():
    import concourse.mybir as mybir
    from concourse import bacc
    from concourse.tile import TileContext

    f32 = mybir.dt.float32
    f16 = mybir.dt.float16
    f8 = mybir.dt.float8e4

    nc = bacc.Bacc()
    xT_d = nc.declare_dram_parameter("xT", [H, S], f16, isOutput=False)
    xn_d = nc.declare_dram_parameter("xn8", [S, MG], f8, isOutput=False)
    wT_d = nc.declare_dram_parameter("wT", [MT * P, KT * P], f16, isOutput=False)
    outT_d = nc.declare_dram_parameter("outT", [MG, S], f16, isOutput=True)

    xT_t = xT_d.rearrange("(kt p) s -> p kt s", p=P)   # [128, 8, 2048]
    xn_t = xn_d.rearrange("(t i p) m -> p t i m", i=2, p=P)  # [128, 8, 2, 512]
    wT_t = wT_d.rearrange("(mt p) (kt m) -> p mt kt m", p=P, m=P)  # [128, 4, 8, 128]

    with TileContext(nc) as tc:
        with (
            tc.tile_pool(name="big", bufs=1) as big,
            tc.tile_pool(name="gp", bufs=1) as gpool,
            tc.tile_pool(name="stage", bufs=8) as stage,
        ):
            xT_sb = big.tile([P, KT, S], f16, tag="xT")
            xn_sb = big.tile([P, SP2, 2, MG], f8, tag="xn")
            wT_sb = big.tile([P, MT, KT, P], f16, tag="wT")
            qT_sb = big.tile([P, MT, S], f16, tag="qT")
            warm_sb = big.tile([P, 512], f16, tag="warm")

            # Pool-queue setup: warmup operand, zeroed G holders, then the
            # non-critical loads (wT m-tiles 1-3, xn8).
            nc.gpsimd.memset(warm_sb, 0.0)
            gbd = []
            for p_i in range(MT):
                g = gpool.tile([P, P], f16, tag=f"g{p_i}", name=f"g{p_i}")
                nc.gpsimd.memset(g, 0.0)
                gbd.append(g)
            # SP queue: everything in strict consumption order so nothing
            # steals HBM bandwidth from the PE-critical xT stream.
            nc.sync.dma_start(out=wT_sb[:, 0], in_=wT_t[:, 0])
            nc.sync.dma_start(out=xT_sb[:, 0], in_=xT_t[:, 0])
            nc.sync.dma_start(out=wT_sb[:, 1], in_=wT_t[:, 1])
            for kt in range(1, KT):
                nc.sync.dma_start(out=xT_sb[:, kt], in_=xT_t[:, kt])
            nc.sync.dma_start(out=xn_sb, in_=xn_t)
            nc.sync.dma_start(out=wT_sb[:, 2:], in_=wT_t[:, 2:])

            def drain_q(mt, psqs):
                for sc in range(SC):
                    dr = nc.vector if sc % 2 == 0 else nc.scalar
                    o = qT_sb[:, mt, sc * 512:(sc + 1) * 512]
                    if dr is nc.vector:
                        dr.tensor_copy(out=o, in_=psqs[sc])
                    else:
                        dr.copy(out=o, in_=psqs[sc])

            def emit_gram(ps_g):
                for p_i in range(MT):
                    psg = ps_g.tile([P, P], f32, tag="psg", name=f"psg{p_i}")
                    cols = slice(p_i * P, (p_i + 1) * P)
                    for t in range(SP2):
                        nc.tensor.matmul(
                            psg,
                            lhsT=xn_sb[:, t, :, cols],
                            rhs=xn_sb[:, t, :, cols],
                            start=(t == 0),
                            stop=(t == SP2 - 1),
                            perf_mode=mybir.MatmulPerfMode.DoubleRow,
                        )
                    nc.scalar.mul(
                        out=gbd[p_i][0:HD, 0:HD], in_=psg[0:HD, 0:HD],
                        mul=1.0 / W_SCALE,
                    )
                    nc.scalar.mul(
                        out=gbd[p_i][HD:P, HD:P], in_=psg[HD:P, HD:P],
                        mul=1.0 / W_SCALE,
                    )

            def emit_out_sc(p_i, sc, ps_o):
                pso = ps_o.tile([P, 512], f32, tag="pso", name=f"pso{p_i}_{sc}")
                nc.tensor.matmul(
                    pso,
                    lhsT=gbd[p_i],
                    rhs=qT_sb[:, p_i, sc * 512:(sc + 1) * 512],
                    start=True,
                    stop=True,
                )
                ot = stage.tile([P, 512], f16, tag="ot", name=f"ot{p_i}_{sc}")
                dr = nc.vector if sc % 2 == 1 else nc.scalar
                if dr is nc.vector:
                    dr.tensor_copy(out=ot, in_=pso)
                else:
                    dr.copy(out=ot, in_=pso)
                st = nc.sync if sc % 2 == 0 else nc.gpsimd
                st.dma_start(
                    out=outT_d[p_i * P:(p_i + 1) * P, sc * 512:(sc + 1) * 512],
                    in_=ot,
                )

            def emit_out(p_i, ps_o):
                for sc in range(SC):
                    emit_out_sc(p_i, sc, ps_o)

            # ---- Phase 1: mt0+mt1 k-outer interleaved over 8 PSUM banks.
            with tc.tile_pool(name="ps1", bufs=2, space="PSUM") as ps1:
                psq01 = [
                    [
                        ps1.tile([P, 512], f32, tag=f"psq{sc}", name=f"psq{mt}_{sc}")
                        for sc in range(SC)
                    ]
                    for mt in range(2)
                ]
                for i in range(N_WARM):
                    nc.tensor.matmul(
                        psq01[0][0],
                        lhsT=warm_sb[:, 0:P],
                        rhs=warm_sb,
                        start=True,
                        stop=True,
                    )
                for kt in range(KT):
                    for mt in range(2):
                        for sc in range(SC):
                            nc.tensor.matmul(
                                psq01[mt][sc],
                                lhsT=wT_sb[:, mt, kt],
                                rhs=xT_sb[:, kt, sc * 512:(sc + 1) * 512],
                                start=(kt == 0),
                                stop=(kt == KT - 1),
                            )
                drain_q(0, psq01[0])
                drain_q(1, psq01[1])

            # ---- Phase 2: gram first, then mt2/mt3 projected sc-major with
            # per-sc drain -> output-matmul -> store chains so the output
            # stage pipelines behind the projection instead of trailing it.
            with (
                tc.tile_pool(name="ps2", bufs=1, space="PSUM") as ps2,
                tc.tile_pool(name="ps_g", bufs=2, space="PSUM") as ps_g,
                tc.tile_pool(name="ps_o", bufs=2, space="PSUM") as ps_o,
            ):
                emit_gram(ps_g)
                emit_out(0, ps_o)
                emit_out(1, ps_o)
                for mt in (2, 3):
                    for sc in range(SC):
                        psq = ps2.tile(
                            [P, 512], f32, tag=f"psq{sc}", name=f"psq{mt}_{sc}"
                        )
                        for kt in range(KT):
                            nc.tensor.matmul(
                                psq,
                                lhsT=wT_sb[:, mt, kt],
                                rhs=xT_sb[:, kt, sc * 512:(sc + 1) * 512],
                                start=(kt == 0),
                                stop=(kt == KT - 1),
                            )
                        dr = nc.vector if sc % 2 == 1 else nc.scalar
                        o = qT_sb[:, mt, sc * 512:(sc + 1) * 512]
                        if dr is nc.vector:
                            dr.tensor_copy(out=o, in_=psq)
                        else:
                            dr.copy(out=o, in_=psq)
                        emit_out_sc(mt, sc, ps_o)
    nc.compile()
    return nc


def _get_nc():
    if "nc" not in _NC_CACHE:
        _NC_CACHE["nc"] = _build_nc()
    return _NC_CACHE["nc"]


def make_in_maps(hidden_states, queries_weight):
    import ml_dtypes

    hs = np.ascontiguousarray(np.asarray(hidden_states, dtype=np.float32))
    w = np.ascontiguousarray(np.asarray(queries_weight, dtype=np.float32))
    in_maps = []
    for c in range(N_CORES):
        b, hg = divmod(c, 2)
        xb = hs[b]
        in_maps.append({
            "xT": np.ascontiguousarray(xb.T).astype(np.float16),
            "xn8": np.ascontiguousarray(xb[:, hg * MG:(hg + 1) * MG]).astype(
                ml_dtypes.float8_e4m3fn
            ),
            "wT": np.ascontiguousarray(
                (w[hg * MG:(hg + 1) * MG, :].T * W_SCALE)
                .reshape(KT, P, MT, P)
                .transpose(2, 1, 0, 3)
                .reshape(MT * P, KT * P)
            ).astype(np.float16),
        })
    return in_maps


def assemble_output(results):
    out = np.empty((B, S, H), dtype=np.float32)
    for c in range(N_CORES):
        b, hg = divmod(c, 2)
        out[b, :, hg * MG:(hg + 1) * MG] = results[c]["outT"].T.astype(np.float32)
    return out


def kernel(hidden_states, queries_weight):
    from concourse.bass_utils import run_bass_kernel_spmd

    in_maps = make_in_maps(hidden_states, queries_weight)
    res = run_bass_kernel_spmd(
        _get_nc(), in_maps, core_ids=list(range(N_CORES))
    ).results
    return assemble_output(res)


if __name__ == "__main__":
    x = np.random.randn(B, S, H).astype(np.float32)
    w = np.random.randn(H, H).astype(np.float32) * 1e-4
    out = kernel(x, w)
    print(out.shape, out.dtype)
